# revision 17
# baseline (speedup 1.0000x reference)
"""Trainium2 Bass kernel for nn_Conv3d_76141180223681.

Computes z = FIR(upsample_conv3d(x, W)) + bias where:
  - upsample_conv3d: lhs-dilated (factor 2) correlation, kernel 3^3, pad 2
    x[8,256,16,16,16] -> y[8,128,33,33,33]
  - FIR: separable depthwise [1,3,3,1]/4 per axis, pad (1,1) -> [8,128,32,32,32]

Strategy: data-parallel over batch N=8 across 8 NeuronCores (weights replicated).
Per core:
  - Polyphase conv: per-axis parity decomposition of the dilated conv
    (even: y[2j] = w0*x[j-1] + w2*x[j]; odd: y[2j+1] = w1*x[j]).
    Along the w axis the FIRST box level of the FIR ([1,3,3,1] = [1,1]^3)
    is folded directly into the conv: the streams
      Sw[j] = yE[j]+yO[j] = w0*x[j-1] + (w1+w2)*x[j]
      Tw[j] = yO[j-1]+yE[j] = (w0+w1)*x[j-1] + w2*x[j]
    are each 2-tap convs, so the tensor engine emits them directly and the
    vector engine skips the whole first w-level. 8 (pd,ph,ps) stream
    volumes, each a sum of small matmuls (fp16 in, fp32 PSUM accumulate);
    contraction over two 128-channel input blocks. Matmuls reading the
    zero d-pad are skipped. x is cast to fp16 and pre-padded on the host.
  - Remaining FIR as box-filter chain in parity space (fp16), with ops
    merged across parities (ph / qw dims) to cut instruction count:
      w: gE[j]=Sw[j]+Tw[j+1]; gO[j]=Tw[j]+Sw[j]; 2zE=gO+gE; 2zO=gE+gO[+1]
      h, d: full 3-level chain (S/T, g, z).
    Engine split: w+h levels on DVE; d-axis levels 1-2 split between
    gpsimd (quarters 0-1) and DVE (quarters 2-3).
  - d-axis final level via identity matmuls on the tensor engine into PSUM;
    scalar engine does bias add + fp32 convert + parity interleave, then DMA.
The 1/64 normalization is folded into the conv weights on host. All stages
are chunked along d (4 output-parity slices per chunk) and software-
pipelined; the Tile framework inserts all synchronization.
"""
import os
import numpy as np

import concourse.bass as bass
import concourse.tile as tile
from concourse import bacc, mybir
from concourse.bass_utils import run_bass_kernel_spmd

N_CORES = 8
P = 128
F16 = mybir.dt.float16
F32 = mybir.dt.float32

NOUT = (17, 16)                      # outputs per axis parity (even, odd)
TAPS = (((0, 0), (1, 2)), ((1, 1),)) # d/h: per parity (base offset, tap idx)
# w-axis streams S,T: (base offset, combined-tap idx); both 17 outputs
TAPS_W = (((0, 0), (1, 1)), ((0, 2), (1, 3)))
CH = 4                               # d-slices per chunk
RY, RZ, RH = 8, 4, 8                 # ring depths: y, zW, zH
CHUNKS = ((0, 4), (4, 8), (8, 12), (12, 16), (16, 17))
NT = 37                              # 3d x 3h x 4w' tap planes + identity


def _chunk_range(c, pd):
    lo, hi = CHUNKS[c]
    return lo, min(hi, NOUT[pd])


def _ring_runs(j_lo, j_hi, deltas, rh):
    """Split [j_lo, j_hi) into runs where (j+d) % rh is contiguous for all d."""
    pts = {j_lo, j_hi}
    for d in deltas:
        j = j_lo + 1
        while j < j_hi:
            if (j + d) % rh == 0:
                pts.add(j)
            j += 1
    pts = sorted(pts)
    return [(a, b) for a, b in zip(pts[:-1], pts[1:])]


def build_program(reps=1):
    nc = bacc.Bacc("TRN2", target_bir_lowering=False, debug=False)

    x_d = nc.dram_tensor("x", [2, P, 16 * 18 * 18], F16, kind="ExternalInput")
    w_d = nc.dram_tensor("w", [2, P, NT * 128], F16, kind="ExternalInput")
    b_d = nc.dram_tensor("b", [P, 1], F32, kind="ExternalInput")
    o_d = nc.dram_tensor("out", [P, 32768], F32, kind="ExternalOutput")

    with tile.TileContext(nc) as tc:
        with (
            tc.tile_pool(name="const", bufs=1) as cpool,
            tc.tile_pool(name="psum", bufs=2, space="PSUM") as ppool,
            tc.tile_pool(name="zpsum", bufs=2, space="PSUM") as zpool,
            tc.tile_pool(name="ftmp", bufs=4) as fpool,
            tc.tile_pool(name="dtmp", bufs=2) as dpool,
            tc.tile_pool(name="dtmpg", bufs=2) as dpool_g,
            tc.tile_pool(name="gtmp", bufs=4) as gpool,
            tc.tile_pool(name="gtmpg", bufs=4) as gpool_g,
            tc.tile_pool(name="ostage", bufs=2) as opool,
        ):
            # ---- persistent tiles ----
            w_sb = cpool.tile([P, 2, NT, 128], F16, name="w_sb")
            bias_sb = cpool.tile([P, 1], F32, name="bias_sb")
            # x fp16, pre-padded on host; no d padding (zero d-taps skipped)
            xq = [cpool.tile([P, 16, 18, 18], F16, name=f"xq{b}") for b in (0, 1)]
            # y[pd]: conv stream volumes [ph, ps(Sw/Tw), ring, h, w]
            y = [cpool.tile([P, 2, 2, RY, 17, 17], F16, name=f"y{pd}")
                 for pd in (0, 1)]
            # zW[pd]: after w-axis FIR [ph, qw, ring, h(18, odd-ph padded), w]
            zW = [cpool.tile([P, 2, 2, RZ, 18, 16], F16, name=f"zw{pd}")
                  for pd in (0, 1)]
            # zH[pd]: after h-axis FIR; payload blocks [qh*2+qw][16b x 16a]
            zH = [cpool.tile([P, RH, 4, 256], F16, name=f"zh{pd}")
                  for pd in (0, 1)]

            # ---- loads ----
            # ordered so chunk-0 pd=1 conv can start earliest: x quarters
            # 0-1 and the td=1 weight planes first, then the rest
            wv = w_d.ap().rearrange("b p (t k) -> b p t k", t=NT, k=128)

            def xdma(blk, qt):
                nc.sync.dma_start(
                    xq[blk][:, 4 * qt:4 * qt + 4],
                    x_d.ap()[blk][:, 1296 * qt:1296 * (qt + 1)]
                    .rearrange("p (d h w) -> p d h w", d=4, h=18, w=18))

            for qt in (0, 1):
                for blk in (0, 1):
                    xdma(blk, qt)
            for blk in (0, 1):
                nc.sync.dma_start(w_sb[:, blk, 12:24], wv[blk][:, 12:24])
            for qt in (2, 3):
                for blk in (0, 1):
                    xdma(blk, qt)
            for blk in (0, 1):
                nc.sync.dma_start(w_sb[:, blk, 0:12], wv[blk][:, 0:12])
            for blk in (0, 1):
                nc.sync.dma_start(w_sb[:, blk, 24:NT], wv[blk][:, 24:NT])
            nc.sync.dma_start(bias_sb[:], b_d.ap())

            # zero: odd-ph zW h-pads, and the odd-ph garbage h-row of y
            # (read by ph-merged ops, never consumed downstream)
            for pd in (0, 1):
                nc.gpsimd.memset(zW[pd][:, 1, :, :, 0:1, :], 0.0)
                nc.gpsimd.memset(zW[pd][:, 1, :, :, 17:18, :], 0.0)
                nc.gpsimd.memset(y[pd][:, 1, :, :, 16:17, :], 0.0)

            def conv_phase(c, pd, ph, ps):
                jlo, jhi = _chunk_range(c, pd)
                nj = jhi - jlo
                if nj <= 0:
                    return
                nh = NOUT[ph]
                nfree = nh * 17
                taps = [(blk, bd, td, bh, th, bw, tw)
                        for blk in (0, 1)
                        for (bd, td) in TAPS[pd]
                        for (bh, th) in TAPS[ph]
                        for (bw, tw) in TAPS_W[ps]]
                for jg in range(0, nj, 2):
                    ng = min(2, nj - jg)
                    pt = ppool.tile([P, 2, 512], F32, name="pt")
                    for jl in range(ng):
                        jd = jlo + jg + jl
                        # drop taps that read the (implicit) zero d-pad
                        jt = [t for t in taps if 0 <= jd + t[1] - 1 < 16]
                        for ti, (blk, bd, td, bh, th, bw, tw) in enumerate(jt):
                            t3 = td * 12 + th * 4 + tw
                            lhsT = w_sb[:, blk, t3]
                            rhs = xq[blk][:, jd + bd - 1,
                                          bh:bh + nh, bw:bw + 17]
                            nc.tensor.matmul(
                                pt[:, jl, 0:nfree], lhsT, rhs,
                                start=(ti == 0), stop=(ti == len(jt) - 1))
                    # evacuate PSUM -> y (fp16)
                    src = pt[:, 0:ng, 0:nfree].rearrange(
                        "p j (h w) -> p j h w", h=nh, w=17)
                    sy = (jlo + jg) % RY
                    dst = y[pd][:, ph, ps, sy:sy + ng, 0:nh, :]
                    nc.scalar.copy(dst, src)

            def w_pass(c, pd):
                # levels 2-3 of the w cascade (level 1 = Sw/Tw done by conv),
                # both h-parities in one op
                jlo, jhi = _chunk_range(c, pd)
                nj = jhi - jlo
                if nj <= 0:
                    return
                add = nc.vector.tensor_add
                sl = slice(0, nj)
                sy = slice(jlo % RY, jlo % RY + nj)
                gE = fpool.tile([P, 2, CH, 17, 16], F16, name="wgE", tag="ftmp")
                gO = fpool.tile([P, 2, CH, 17, 17], F16, name="wgO", tag="ftmp")
                add(gE[:, :, sl], y[pd][:, :, 0, sy, :, 0:16],
                    y[pd][:, :, 1, sy, :, 1:17])
                add(gO[:, :, sl], y[pd][:, :, 1, sy, :, :],
                    y[pd][:, :, 0, sy, :, :])
                for ph in (0, 1):
                    nh, ho = NOUT[ph], ph
                    add(zW[pd][:, ph, 0, sl, ho:ho + nh, :],
                        gO[:, ph, sl, 0:nh, 0:16], gE[:, ph, sl, 0:nh, :])
                    add(zW[pd][:, ph, 1, sl, ho:ho + nh, :],
                        gE[:, ph, sl, 0:nh, :], gO[:, ph, sl, 0:nh, 1:17])

            def h_pass(c, pd, l0=0, l1=CH):
                # full h cascade, both w-quarters in one op
                jlo, jhi = _chunk_range(c, pd)
                jlo, jhi = jlo + l0, min(jlo + l1, jhi)
                nj = jhi - jlo
                if nj <= 0:
                    return
                add = nc.vector.tensor_add
                sl = slice(l0, l0 + nj)
                Sh = fpool.tile([P, 2, CH, 17, 16], F16, name="hS", tag="ftmp")
                Th = fpool.tile([P, 2, CH, 17, 16], F16, name="hT", tag="ftmp")
                gEh = fpool.tile([P, 2, CH, 17, 16], F16, name="hgE", tag="ftmp")
                gOh = fpool.tile([P, 2, CH, 17, 16], F16, name="hgO", tag="ftmp")
                add(Sh[:, :, sl], zW[pd][:, 0, :, sl, 0:17, :],
                    zW[pd][:, 1, :, sl, 1:18, :])
                add(Th[:, :, sl], zW[pd][:, 1, :, sl, 0:17, :],
                    zW[pd][:, 0, :, sl, 0:17, :])
                add(gEh[:, :, sl, 0:16, :], Sh[:, :, sl, 0:16, :],
                    Th[:, :, sl, 1:17, :])
                add(gOh[:, :, sl], Th[:, :, sl], Sh[:, :, sl])
                s0 = jlo % RH
                o0 = zH[pd][:, s0:s0 + nj, 0:2, :].rearrange(
                    "p s q (b a) -> p q s b a", b=16, a=16)
                o1 = zH[pd][:, s0:s0 + nj, 2:4, :].rearrange(
                    "p s q (b a) -> p q s b a", b=16, a=16)
                add(o0, gOh[:, :, sl, 0:16, :], gEh[:, :, sl, 0:16, :])
                add(o1, gEh[:, :, sl, 0:16, :], gOh[:, :, sl, 1:17, :])

            def d_pass_levels12(e, q, eng, stpool, ggpool):
                # levels 1-2 of the d-axis cascade for one w/h quarter q:
                # temps over j in [4e, 4e+5)
                jl = 4 * e
                E, O = zH[0], zH[1]
                q2 = slice(q, q + 1)
                tadd, tcp = eng.tensor_add, eng.tensor_copy
                Sd = stpool.tile([P, 5, 1, 256], F16, name="dS", tag="dtmp")
                Td = stpool.tile([P, 5, 1, 256], F16, name="dT", tag="dtmp")
                gEd = ggpool.tile([P, 5, 1, 256], F16, name="dgE", tag="gtmp")
                gOd = ggpool.tile([P, 5, 1, 256], F16, name="dgO", tag="gtmp")
                # S[j] = E[j] + O[j] for j < 16; S[16] = E[16]
                s_hi = min(jl + 5, 16)
                for (a, b) in _ring_runs(jl, s_hi, (0,), RH):
                    sa = a % RH
                    tadd(Sd[:, a - jl:b - jl], E[:, sa:sa + (b - a), q2, :],
                         O[:, sa:sa + (b - a), q2, :])
                if jl + 5 > 16:  # S[16] = E[16]
                    tcp(Sd[:, 16 - jl:17 - jl], E[:, 16 % RH:16 % RH + 1, q2, :])
                # Tp[j] = O[j-1] + E[j]; Tp[0] = E[0]
                t_lo = jl
                if e == 0:
                    tcp(Td[:, 0:1], E[:, 0:1, q2, :])
                    t_lo = 1
                for (a, b) in _ring_runs(t_lo, jl + 5, (0, -1), RH):
                    sa, so = a % RH, (a - 1) % RH
                    tadd(Td[:, a - jl:b - jl], O[:, so:so + (b - a), q2, :],
                         E[:, sa:sa + (b - a), q2, :])
                tadd(gEd[:, 0:4], Sd[:, 0:4], Td[:, 1:5])
                tadd(gOd[:, 0:5], Td[:, 0:5], Sd[:, 0:5])
                return gEd, gOd

            ident = w_sb[:, 0, NT - 1]  # identity matrix tap

            ov = o_d.ap().rearrange("p (c k q s) -> p c k q s",
                                    c=4, k=4, q=2, s=1024)

            def d_mms_part(e, q, gEd, gOd, ost):
                for qd in (0, 1):
                    d_mms_one(e, q, qd, gEd, gOd, ost)

            def d_mms_one(e, q, qd, gEd, gOd, ost):
                # level 3 on the tensor engine via identity matmuls into
                # PSUM; the final pass (scalar) reads PSUM, adds bias,
                # converts to f32 and interleaves into the output layout.
                qh, qw = q >> 1, q & 1
                if True:
                    pz = zpool.tile([P, 4, 256], F32, name="pz")
                    for ap in (0, 2):  # a-pairs
                        if qd == 0:  # zE[a] = gO[a] + gE[a]
                            r0 = gOd[:, ap:ap + 2, 0, :]
                            r1 = gEd[:, ap:ap + 2, 0, :]
                        else:        # zO[a] = gE[a] + gO[a+1]
                            r0 = gEd[:, ap:ap + 2, 0, :]
                            r1 = gOd[:, ap + 1:ap + 3, 0, :]
                        out = pz[:, ap:ap + 2, :]
                        nc.tensor.matmul(out, ident, r0,
                                         start=True, stop=False)
                        nc.tensor.matmul(out, ident, r1,
                                         start=False, stop=True)
                    # final: bias + f32 + (h,w) interleave, PSUM -> SBUF
                    src = pz[:, 0:4, :].rearrange(
                        "p k (b a) -> p k b a", b=16, a=16)
                    dst = ost[qd][:].rearrange(
                        "p k (b g a w) -> p k b g a w",
                        b=16, g=2, a=16, w=2)[:, :, :, qh, :, qw]
                    nc.scalar.add(dst, src, bias_sb[:, 0:1])

            live = {}

            def d_chunk_main(e, all_dve=False):
                # levels only: the z-matmuls would head-of-line block the
                # PE queue (waiting for these levels) and so stall the next
                # chunk's conv — they run a full chunk later (d_chunk_lagged)
                gs = {}
                for q in (2, 3):
                    gs[q] = d_pass_levels12(e, q, nc.vector, dpool, gpool)
                for q in (0, 1):
                    eng = nc.vector if all_dve else nc.gpsimd
                    gs[q] = d_pass_levels12(e, q, eng, dpool_g, gpool_g)
                live[e] = gs

            def d_chunk_lagged(e):
                gs = live.pop(e)
                ost = [opool.tile([P, 4, 1024], F32, name=f"ost{qd}",
                                  tag="ost") for qd in (0, 1)]
                for qd in (0, 1):
                    for q in (0, 1, 2, 3):
                        d_mms_one(e, q, qd, gs[q][0], gs[q][1], ost)
                    nc.sync.dma_start(ov[:, e, :, qd, :], ost[qd][:])

            # ---- main pipeline ----
            def pipeline():
                for c in range(5):
                    for pd in (1, 0):
                        for ph in (0, 1):
                            for ps in (0, 1):
                                conv_phase(c, pd, ph, ps)
                    w_pass(c, 1)
                    w_pass(c, 0)
                    # h-pass first j-slices early: the lagged d-pass needs
                    # only those zH writes (halo); the remaining j-slices run
                    # after the d-pass (their writes would clobber zH ring
                    # slots the d-pass halo-reads)
                    h_pass(c, 0, 0, 1)
                    h_pass(c, 1, 0, 1)
                    if c >= 2:
                        d_chunk_lagged(c - 2)
                    if c >= 1:
                        d_chunk_main(c - 1, all_dve=(c == 4))
                    h_pass(c, 0, 1, CH)
                    h_pass(c, 1, 1, CH)
                d_chunk_lagged(3)

            if reps == 1:
                pipeline()
            else:
                with tc.For_i(0, reps, 1):
                    pipeline()

    nc.compile()
    return nc


_NC = None


def _get_program():
    global _NC
    if _NC is None:
        _NC = build_program()
    return _NC


def _prep_inputs(x, weight, bias):
    # weights: fold 1/64 FIR normalization; [cout,cin,3,3,3] ->
    # [cin, d, h, w, cout]; build 4 combined w-axis tap planes
    # {w0, w1+w2, w0+w1, w2} -> [2 blk, 128, 37*128] fp16 (+ identity)
    w = (np.asarray(weight, dtype=np.float32) / 64.0)
    w = w.transpose(1, 2, 3, 4, 0)  # [256, 3, 3, 3, 128]
    wt = np.zeros((256, 3, 3, 4, 128), dtype=np.float32)
    wt[:, :, :, 0] = w[:, :, :, 0]
    wt[:, :, :, 1] = w[:, :, :, 1] + w[:, :, :, 2]
    wt[:, :, :, 2] = w[:, :, :, 0] + w[:, :, :, 1]
    wt[:, :, :, 3] = w[:, :, :, 2]
    wNT = np.zeros((2, 128, NT, 128), dtype=np.float32)
    wNT[:, :, 0:36, :] = wt.reshape(2, 128, 36, 128)
    wNT[0, :, NT - 1, :] = np.eye(128, dtype=np.float32)
    w_host = np.ascontiguousarray(
        wNT.reshape(2, 128, NT * 128)).astype(np.float16)
    b_host = np.ascontiguousarray(
        np.asarray(bias, dtype=np.float32).reshape(P, 1))
    xp = np.zeros((N_CORES, 2, 128, 16, 18, 18), dtype=np.float16)
    xr = np.asarray(x, dtype=np.float32).reshape(N_CORES, 2, 128, 16, 16, 16)
    xp[:, :, :, :, 1:17, 1:17] = xr.astype(np.float16)
    in_maps = []
    for n in range(N_CORES):
        xn = np.ascontiguousarray(xp[n].reshape(2, 128, 16 * 18 * 18))
        in_maps.append({"x": xn, "w": w_host, "b": b_host})
    return in_maps


def run(x, weight, bias, trace=False):
    nc = _get_program()
    in_maps = _prep_inputs(x, weight, bias)
    res = run_bass_kernel_spmd(nc, in_maps, list(range(N_CORES)), trace=trace)
    out = np.stack([res.results[n]["out"].reshape(P, 32, 32, 32)
                    for n in range(N_CORES)])
    return out, res


def kernel(x, weight, bias):
    out, _ = run(x, weight, bias, trace=False)
    return out


if __name__ == "__main__":
    nc = build_program()
    print("built ok; instructions:",
          sum(len(b.instructions) for b in nc.main_func.blocks))


# revision 18
# speedup vs baseline: 1.1016x; 1.1016x over previous
"""Trainium2 Bass kernel for nn_Conv3d_76141180223681.

Computes z = FIR(upsample_conv3d(x, W)) + bias where:
  - upsample_conv3d: lhs-dilated (factor 2) correlation, kernel 3^3, pad 2
    x[8,256,16,16,16] -> y[8,128,33,33,33]
  - FIR: separable depthwise [1,3,3,1]/4 per axis, pad (1,1) -> [8,128,32,32,32]

Strategy: data-parallel over batch N=8 across 8 NeuronCores (weights replicated).
Per core:
  - Polyphase conv: per-axis parity decomposition of the dilated conv. For
    upsampled index s (parity p, j=s//2): even: y[2j] = w0*x[j-1] + w2*x[j];
    odd: y[2j+1] = w1*x[j]. 8 (pd,ph,pw) phase volumes, each computed as a sum
    of small matmuls on the tensor engine (fp16 in, fp32 PSUM accumulate);
    contraction over 128-channel input blocks. Matmuls whose input d-slice is
    the zero pad are skipped entirely (xq carries no d padding).
  - FIR as a box-filter chain in parity space (fp16):
    per axis with parity streams E (17) and O (16):
      S[j]=E[j]+O[j]; Tp[j]=O[j-1]+E[j]; gE[j]=S[j]+Tp[j+1]; gOp[j]=Tp[j]+S[j];
      4*zE[a]=gOp[a]+gE[a]; 4*zO[a]=gE[a]+gOp[a+1]
    The 1/64 (= (1/4)^3) normalization is folded into the conv weights on host.
    Engine split: w-axis + h-axis(pd=0) + d-axis levels run on the vector
    engine (DVE); h-axis(pd=1) runs on gpsimd (Pool) in parallel.
  - d-axis final level via identity matmuls on the tensor engine into PSUM;
    scalar engine does bias add + fp32 convert + parity interleave, then DMA.
All stages are chunked along d (4 output-parity slices per chunk) and
software-pipelined; the Tile framework inserts all synchronization.
"""
import os
import numpy as np

import concourse.bass as bass
import concourse.tile as tile
from concourse import bacc, mybir
from concourse.bass_utils import run_bass_kernel_spmd

N_CORES = 8
P = 128
F16 = mybir.dt.float16
F32 = mybir.dt.float32

NOUT = (17, 16)                      # outputs per axis parity (even, odd)
TAPS = (((0, 0), (1, 2)), ((1, 1),)) # per parity: (xpad base offset, tap idx)
CH = 4                               # d-slices per chunk
RY, RZ, RH = 8, 4, 8                 # ring depths: y, zW, zH
CHUNKS = ((0, 4), (4, 8), (8, 12), (12, 16), (16, 17))


def _chunk_range(c, pd):
    lo, hi = CHUNKS[c]
    return lo, min(hi, NOUT[pd])


def _ring_runs(j_lo, j_hi, deltas, rh):
    """Split [j_lo, j_hi) into runs where (j+d) % rh is contiguous for all d."""
    pts = {j_lo, j_hi}
    for d in deltas:
        j = j_lo + 1
        while j < j_hi:
            if (j + d) % rh == 0:
                pts.add(j)
            j += 1
    pts = sorted(pts)
    return [(a, b) for a, b in zip(pts[:-1], pts[1:])]


def build_program(reps=1):
    nc = bacc.Bacc("TRN2", target_bir_lowering=False, debug=False)

    x_d = nc.dram_tensor("x", [2, P, 4096], F32, kind="ExternalInput")
    w_d = nc.dram_tensor("w", [2, P, 28 * 128], F16, kind="ExternalInput")
    b_d = nc.dram_tensor("b", [P, 1], F32, kind="ExternalInput")
    o_d = nc.dram_tensor("out", [P, 32768], F32, kind="ExternalOutput")

    with tile.TileContext(nc) as tc:
        with (
            tc.tile_pool(name="const", bufs=1) as cpool,
            tc.tile_pool(name="stage", bufs=2) as spool,
            tc.tile_pool(name="psum", bufs=2, space="PSUM") as ppool,
            tc.tile_pool(name="zpsum", bufs=2, space="PSUM") as zpool,
            tc.tile_pool(name="wtmp", bufs=6) as wpool,
            tc.tile_pool(name="hptmp", bufs=4) as hpool2,
            tc.tile_pool(name="dtmp", bufs=2) as dpool,
            tc.tile_pool(name="gtmp", bufs=3) as gpool,
            tc.tile_pool(name="ostage", bufs=2) as opool,
        ):
            # ---- persistent tiles ----
            w_sb = cpool.tile([P, 2, 28, 128], F16, name="w_sb")
            bias_sb = cpool.tile([P, 1], F32, name="bias_sb")
            # x in fp16; no d padding (zero-pad d taps are skipped), h/w padded
            xq = [cpool.tile([P, 16, 18, 18], F16, name=f"xq{b}") for b in (0, 1)]
            # y[pd][ph][pw]: conv output parity volumes, ring over jd
            y = [[[cpool.tile([P, RY, NOUT[ph], 17 + pw], F16,
                              name=f"y{pd}{ph}{pw}")
                   for pw in (0, 1)] for ph in (0, 1)] for pd in (0, 1)]
            # zW[pd][ph][qw]: after w-axis FIR; odd-ph is h-phys-padded (18 rows)
            zW = [[[cpool.tile([P, RZ, 17 + ph, 16], F16,
                               name=f"zw{pd}{ph}{qw}")
                    for qw in (0, 1)] for ph in (0, 1)] for pd in (0, 1)]
            # zH[pd]: after h-axis FIR; payload blocks [qh*2+qw][16b x 16a]
            zH = [cpool.tile([P, RH, 4, 256], F16, name=f"zh{pd}")
                  for pd in (0, 1)]

            # ---- loads ----
            for blk in (0, 1):
                nc.sync.dma_start(w_sb[:, blk], w_d.ap()[blk])
            nc.sync.dma_start(bias_sb[:], b_d.ap())

            # zero x h/w pad shell (no d faces needed) and only the pad cells
            # of the parity-padded buffers (tiny strips); all on gpsimd which
            # is idle at start
            for blk in (0, 1):
                nc.gpsimd.memset(xq[blk][:, :, 0:1, :], 0.0)
                nc.gpsimd.memset(xq[blk][:, :, 17:18, :], 0.0)
                nc.gpsimd.memset(xq[blk][:, :, 1:17, 0:1], 0.0)
                nc.gpsimd.memset(xq[blk][:, :, 1:17, 17:18], 0.0)
            for pd in (0, 1):
                for ph in (0, 1):
                    nc.gpsimd.memset(y[pd][ph][1][:, :, :, 0:1], 0.0)
                    nc.gpsimd.memset(y[pd][ph][1][:, :, :, 17:18], 0.0)
                for qw in (0, 1):
                    nc.gpsimd.memset(zW[pd][1][qw][:, :, 0:1, :], 0.0)
                    nc.gpsimd.memset(zW[pd][1][qw][:, :, 17:18, :], 0.0)

            # x load (f32) + fp16 convert into padded interior on the scalar
            # engine, 2 d-slices at a time, both channel blocks interleaved so
            # chunk-0 conv can start after the first few groups
            for g in range(8):
                for blk in (0, 1):
                    st = spool.tile([P, 512], F32, name="xstage")
                    nc.sync.dma_start(
                        st[:], x_d.ap()[blk][:, 512 * g:512 * (g + 1)])
                    src = st[:].rearrange("p (d h w) -> p d h w",
                                          d=2, h=16, w=16)
                    dst = xq[blk][:, 2 * g:2 * g + 2, 1:17, 1:17]
                    nc.scalar.copy(dst, src)

            def conv_phase(c, pd, ph, pw):
                jlo, jhi = _chunk_range(c, pd)
                nj = jhi - jlo
                if nj <= 0:
                    return
                nh, nw = NOUT[ph], NOUT[pw]
                nfree = nh * nw
                taps = [(blk, bd, td, bh, th, bw, tw)
                        for blk in (0, 1)
                        for (bd, td) in TAPS[pd]
                        for (bh, th) in TAPS[ph]
                        for (bw, tw) in TAPS[pw]]
                for jg in range(0, nj, 2):
                    ng = min(2, nj - jg)
                    pt = ppool.tile([P, 2, 512], F32, name="pt")
                    for jl in range(ng):
                        jd = jlo + jg + jl
                        # drop taps that read the (implicit) zero d-pad
                        jt = [t for t in taps if 0 <= jd + t[1] - 1 < 16]
                        for ti, (blk, bd, td, bh, th, bw, tw) in enumerate(jt):
                            t3 = td * 9 + th * 3 + tw
                            lhsT = w_sb[:, blk, t3]
                            rhs = xq[blk][:, jd + bd - 1,
                                          bh:bh + nh, bw:bw + nw]
                            nc.tensor.matmul(
                                pt[:, jl, 0:nfree], lhsT, rhs,
                                start=(ti == 0), stop=(ti == len(jt) - 1))
                    # evacuate PSUM -> y (fp16)
                    src = pt[:, 0:ng, 0:nfree].rearrange(
                        "p j (h w) -> p j h w", h=nh, w=nw)
                    sy = (jlo + jg) % RY
                    dst = y[pd][ph][pw][:, sy:sy + ng, :, pw:pw + nw]
                    nc.scalar.copy(dst, src)

            def w_pass(c, pd, ph):
                jlo, jhi = _chunk_range(c, pd)
                nj = jhi - jlo
                if nj <= 0:
                    return
                add = nc.vector.tensor_add
                nh = NOUT[ph]
                yE, yO = y[pd][ph][0], y[pd][ph][1]
                sl = slice(0, nj)
                sy = slice(jlo % RY, jlo % RY + nj)
                S = wpool.tile([P, CH, 17, 17], F16, name="wS", tag="wtmp")
                Tp = wpool.tile([P, CH, 17, 17], F16, name="wT", tag="wtmp")
                gE = wpool.tile([P, CH, 17, 17], F16, name="wgE", tag="wtmp")
                gO = wpool.tile([P, CH, 17, 17], F16, name="wgO", tag="wtmp")
                add(S[:, sl, 0:nh, 0:17], yE[:, sy, :, 0:17], yO[:, sy, :, 1:18])
                add(Tp[:, sl, 0:nh, 0:17], yO[:, sy, :, 0:17], yE[:, sy, :, 0:17])
                add(gE[:, sl, 0:nh, 0:16], S[:, sl, 0:nh, 0:16], Tp[:, sl, 0:nh, 1:17])
                add(gO[:, sl, 0:nh, 0:17], Tp[:, sl, 0:nh, 0:17], S[:, sl, 0:nh, 0:17])
                ho = ph  # odd-ph zW buffers are h-phys-padded by 1
                add(zW[pd][ph][0][:, sl, ho:ho + nh, :],
                    gO[:, sl, 0:nh, 0:16], gE[:, sl, 0:nh, 0:16])
                add(zW[pd][ph][1][:, sl, ho:ho + nh, :],
                    gE[:, sl, 0:nh, 0:16], gO[:, sl, 0:nh, 1:17])

            def h_pass(c, pd, qw, l0=0, l1=CH, eng=None, pool=None, tag=None):
                jlo, jhi = _chunk_range(c, pd)
                jlo, jhi = jlo + l0, min(jlo + l1, jhi)
                nj = jhi - jlo
                if nj <= 0:
                    return
                add = eng.tensor_add
                zWE, zWO = zW[pd][0][qw], zW[pd][1][qw]
                sl = slice(l0, l0 + nj)
                Sh = pool.tile([P, CH, 17, 17], F16, name="hS", tag=tag)
                Th = pool.tile([P, CH, 17, 17], F16, name="hT", tag=tag)
                gEh = pool.tile([P, CH, 17, 17], F16, name="hgE", tag=tag)
                gOh = pool.tile([P, CH, 17, 17], F16, name="hgO", tag=tag)
                add(Sh[:, sl, 0:17, 0:16], zWE[:, sl, 0:17, :], zWO[:, sl, 1:18, :])
                add(Th[:, sl, 0:17, 0:16], zWO[:, sl, 0:17, :], zWE[:, sl, 0:17, :])
                add(gEh[:, sl, 0:16, 0:16], Sh[:, sl, 0:16, 0:16], Th[:, sl, 1:17, 0:16])
                add(gOh[:, sl, 0:17, 0:16], Th[:, sl, 0:17, 0:16], Sh[:, sl, 0:17, 0:16])
                s0 = jlo % RH
                o0 = zH[pd][:, s0:s0 + nj, 0 + qw, :].rearrange(
                    "p s (b a) -> p s b a", b=16, a=16)
                o1 = zH[pd][:, s0:s0 + nj, 2 + qw, :].rearrange(
                    "p s (b a) -> p s b a", b=16, a=16)
                add(o0, gOh[:, sl, 0:16, 0:16], gEh[:, sl, 0:16, 0:16])
                add(o1, gEh[:, sl, 0:16, 0:16], gOh[:, sl, 1:17, 0:16])

            def h_dve(c, pd, qw, l0=0, l1=CH):
                h_pass(c, pd, qw, l0, l1, eng=nc.vector, pool=wpool, tag="wtmp")

            def h_pool(c, pd, qw, l0=0, l1=CH):
                h_pass(c, pd, qw, l0, l1, eng=nc.gpsimd, pool=hpool2, tag="hptmp")

            def d_pass_levels12(e, qp):
                # levels 1-2 of the d-axis cascade for quarter pair qp
                # (quarters 2qp, 2qp+1): temps over j in [4e, 4e+5)
                jl = 4 * e
                E, O = zH[0], zH[1]
                q2 = slice(2 * qp, 2 * qp + 2)
                tadd, tcp = nc.vector.tensor_add, nc.vector.tensor_copy
                Sd = dpool.tile([P, 5, 2, 256], F16, name="dS", tag="dtmp")
                Td = dpool.tile([P, 5, 2, 256], F16, name="dT", tag="dtmp")
                gEd = gpool.tile([P, 5, 2, 256], F16, name="dgE", tag="gtmp")
                gOd = gpool.tile([P, 5, 2, 256], F16, name="dgO", tag="gtmp")
                # S[j] = E[j] + O[j] for j < 16; S[16] = E[16]
                s_hi = min(jl + 5, 16)
                for (a, b) in _ring_runs(jl, s_hi, (0,), RH):
                    sa = a % RH
                    tadd(Sd[:, a - jl:b - jl], E[:, sa:sa + (b - a), q2, :],
                         O[:, sa:sa + (b - a), q2, :])
                if jl + 5 > 16:  # S[16] = E[16]
                    tcp(Sd[:, 16 - jl:17 - jl], E[:, 16 % RH:16 % RH + 1, q2, :])
                # Tp[j] = O[j-1] + E[j]; Tp[0] = E[0]
                t_lo = jl
                if e == 0:
                    tcp(Td[:, 0:1], E[:, 0:1, q2, :])
                    t_lo = 1
                for (a, b) in _ring_runs(t_lo, jl + 5, (0, -1), RH):
                    sa, so = a % RH, (a - 1) % RH
                    tadd(Td[:, a - jl:b - jl], O[:, so:so + (b - a), q2, :],
                         E[:, sa:sa + (b - a), q2, :])
                tadd(gEd[:, 0:4], Sd[:, 0:4], Td[:, 1:5])
                tadd(gOd[:, 0:5], Td[:, 0:5], Sd[:, 0:5])
                return gEd, gOd

            ident = w_sb[:, 0, 27]  # identity matrix tap

            ov = o_d.ap().rearrange("p (c k q s) -> p c k q s",
                                    c=4, k=4, q=2, s=1024)

            def d_mms_part(e, qp, gEd, gOd, ost):
                # level 3 on the tensor engine via identity matmuls into
                # PSUM; the final pass (scalar) reads PSUM, adds bias,
                # converts to f32 and interleaves into the output layout.
                for qi in (0, 1):
                    q = 2 * qp + qi
                    qh, qw = q >> 1, q & 1
                    for qd in (0, 1):
                        pz = zpool.tile([P, 4, 256], F32, name="pz")
                        for ap in (0, 2):  # a-pairs
                            if qd == 0:  # zE[a] = gO[a] + gE[a]
                                r0 = gOd[:, ap:ap + 2, qi, :]
                                r1 = gEd[:, ap:ap + 2, qi, :]
                            else:        # zO[a] = gE[a] + gO[a+1]
                                r0 = gEd[:, ap:ap + 2, qi, :]
                                r1 = gOd[:, ap + 1:ap + 3, qi, :]
                            out = pz[:, ap:ap + 2, :]
                            nc.tensor.matmul(out, ident, r0,
                                             start=True, stop=False)
                            nc.tensor.matmul(out, ident, r1,
                                             start=False, stop=True)
                        # final: bias + f32 + (h,w) interleave, PSUM -> SBUF
                        src = pz[:, 0:4, :].rearrange(
                            "p k (b a) -> p k b a", b=16, a=16)
                        dst = ost[qd][:].rearrange(
                            "p k (b g a w) -> p k b g a w",
                            b=16, g=2, a=16, w=2)[:, :, :, qh, :, qw]
                        nc.scalar.add(dst, src, bias_sb[:, 0:1])

            def d_chunk(e):
                ost = [opool.tile([P, 4, 1024], F32, name=f"ost{qd}",
                                  tag="ost") for qd in (0, 1)]
                for qp in (0, 1):
                    gEd, gOd = d_pass_levels12(e, qp)
                    d_mms_part(e, qp, gEd, gOd, ost)
                for qd in (0, 1):
                    nc.sync.dma_start(ov[:, e, :, qd, :], ost[qd][:])

            # ---- main pipeline ----
            def pipeline():
                for c in range(5):
                    # conv: pd=1 phases first so the pd=1 FIR chain
                    # (DVE w-pass -> gpsimd h-pass) starts early
                    for pd in (1, 0):
                        for ph in (0, 1):
                            for pw in (0, 1):
                                conv_phase(c, pd, ph, pw)
                    for ph in (0, 1):
                        w_pass(c, 1, ph)
                    # gpsimd h-pass pd=1: first j-slice early (the lagged
                    # d-pass needs its zH writes), last j-slice after the
                    # d-pass (it would clobber a zH ring slot the d-pass
                    # halo-reads)
                    for qw in (0, 1):
                        h_pool(c, 1, qw, 0, 1)
                    for qw in (0, 1):
                        h_pool(c, 1, qw, 1, 3)
                    for ph in (0, 1):
                        w_pass(c, 0, ph)
                    for qw in (0, 1):
                        h_dve(c, 0, qw)
                    if c >= 1:
                        d_chunk(c - 1)
                    for qw in (0, 1):
                        h_pool(c, 1, qw, 3, CH)

            if reps == 1:
                pipeline()
            else:
                with tc.For_i(0, reps, 1):
                    pipeline()

    nc.compile()
    return nc


_NC = None


def _get_program():
    global _NC
    if _NC is None:
        _NC = build_program()
    return _NC


def _prep_inputs(x, weight, bias):
    # weights: fold 1/64 FIR normalization; [cout,cin,3,3,3] ->
    # [cin, tap, cout] -> [2 blk, 128, 28*128] fp16
    w = (np.asarray(weight, dtype=np.float32) / 64.0)
    w = w.transpose(1, 2, 3, 4, 0).reshape(256, 27, 128)
    w28 = np.zeros((2, 128, 28, 128), dtype=np.float32)
    w28[:, :, 0:27, :] = w.reshape(2, 128, 27, 128)
    w28[0, :, 27, :] = np.eye(128, dtype=np.float32)
    w_host = np.ascontiguousarray(w28.reshape(2, 128, 28 * 128)).astype(np.float16)
    b_host = np.ascontiguousarray(np.asarray(bias, dtype=np.float32).reshape(P, 1))
    in_maps = []
    for n in range(N_CORES):
        xn = np.ascontiguousarray(
            np.asarray(x[n], dtype=np.float32).reshape(2, 128, 4096))
        in_maps.append({"x": xn, "w": w_host, "b": b_host})
    return in_maps


def run(x, weight, bias, trace=False):
    nc = _get_program()
    in_maps = _prep_inputs(x, weight, bias)
    res = run_bass_kernel_spmd(nc, in_maps, list(range(N_CORES)), trace=trace)
    out = np.stack([res.results[n]["out"].reshape(P, 32, 32, 32)
                    for n in range(N_CORES)])
    return out, res


def kernel(x, weight, bias):
    out, _ = run(x, weight, bias, trace=False)
    return out


if __name__ == "__main__":
    nc = build_program()
    print("built ok; instructions:",
          sum(len(b.instructions) for b in nc.main_func.blocks))
